# revision 2
# baseline (speedup 1.0000x reference)
"""BiLSTM classifier Trainium2 kernel.

Data-parallel over batch across 8 NeuronCores: each core runs the full
BiLSTM (fwd LSTM, bwd LSTM, 2nd LSTM, classifier head) for its 32-row
batch shard, with replicated weights. Matmuls in bf16 (fp32 PSUM
accumulate); hh weight streams optionally fp8-e4m3 with DoubleRow.

Layout notes (per core, B=32 local batch):
- Recurrent matmuls keep the batch as the PE stationary operand
  (lhsT = h^T [K-tile, 32]) and stream the weights as the moving
  operand, so per-step PE time ~ (#weight columns) regardless of M.
- Gates are reordered [g|i|f|o] (host-side weight row permutation) so
  the tanh(g) input is ready earliest and o (needed last) comes last.
- h is transposed each step via PE-transpose (identity matmul) to feed
  the next step's stationary operand; c stays in batch-major layout.
- X := x @ W_ih^T + b precomputed for all timesteps (fully parallel),
  added to the hh-matmul PSUM via identity-matmul accumulation (layer 1)
  or DVE add (layer 2).
- DRAM intermediates are allocated per-m-tile (4 timesteps) so the Tile
  scheduler can overlap phases across the per-step dependency chain.
"""

import sys

sys.path.insert(0, "/opt/trn_rl_repo")

import numpy as np
import ml_dtypes

import concourse.bass as bass
import concourse.mybir as mybir
import concourse.tile as tile
from concourse import bacc
from concourse.bass_utils import run_bass_kernel_spmd

AF = mybir.ActivationFunctionType
BF16 = mybir.dt.bfloat16
F32 = mybir.dt.float32
F8 = mybir.dt.float8e4

B, D, H = 256, 256, 512
H2 = 2 * H          # 1024 second-layer hidden
L = 2
NCORES = 8
BL = B // NCORES    # 32 local batch

ALL_PHASES = ("ih", "A", "Xs", "B")


def _emit_ih(nc, tc, T, t_, g):
    """X{f,b}[mt] = x_(4t) @ W_ih^T + b for all m-tiles."""
    MT = T // 4
    xT = t_["xT"]
    with tc.tile_pool(name="ihw", bufs=1) as ihw, \
         tc.tile_pool(name="ihx", bufs=3) as ihx, \
         tc.tile_pool(name="ihp", bufs=2, space="PSUM") as ihp, \
         tc.tile_pool(name="iho", bufs=3) as iho:
        w_sb = {}
        b_sb = {}
        for d, wn, bn in (("f", "wf_ih", "bf_r"), ("b", "wb_ih", "bb_r")):
            w_sb[d] = ihw.tile([128, 2, 4 * H], BF16, tag=f"wih_{d}", name=f"wih_{d}")
            nc.sync.dma_start(out=w_sb[d], in_=t_[wn].rearrange("(k p) n -> p k n", p=128))
            b_sb[d] = ihw.tile([1, 4 * H], BF16, tag=f"bih_{d}", name=f"bih_{d}")
            nc.sync.dma_start(out=b_sb[d], in_=t_[bn][:, :])

        for mt in range(MT):
            t0 = mt * 4
            xt = ihx.tile([128, 2, 4, 32], BF16, tag="xt", name="xt")
            nc.sync.dma_start(
                out=xt,
                in_=xT.rearrange("(k p) t b -> p k t b", p=128)[:, :, t0:t0 + 4, :],
            )
            for d in ("f", "b"):
                ps = ihp.tile([128, 4 * H], F32, tag="ps", name="ps")
                for nh in range(4):
                    nsl = slice(nh * 512, (nh + 1) * 512)
                    for kt in range(2):
                        nc.tensor.matmul(
                            ps[:, nsl],
                            xt[:, kt].rearrange("p t b -> p (t b)"),
                            w_sb[d][:, kt, nsl],
                            start=(kt == 0), stop=False,
                        )
                    nc.tensor.matmul(
                        ps[:, nsl], t_["ones_t"][:, :128], b_sb[d][:, nsl],
                        start=False, stop=True,
                    )
                ot = iho.tile([128, 4 * H], BF16, tag="ot", name="ot")
                nc.vector.tensor_copy(ot, ps)
                Xd = g["Xf"] if d == "f" else g["Xb"]
                nc.sync.dma_start(
                    out=Xd[mt].rearrange("t b n -> (t b) n"), in_=ot,
                )


def _emit_A(nc, tc, T, t_, g, use_fp8):
    """fwd+bwd layer-1 recurrences, interleaved per step."""
    hdt = F8 if use_fp8 else BF16
    with tc.tile_pool(name="aw", bufs=1) as aw, \
         tc.tile_pool(name="ax", bufs=3) as ax, \
         tc.tile_pool(name="ag", bufs=3, space="PSUM") as ag, \
         tc.tile_pool(name="atr", bufs=1, space="PSUM") as atr, \
         tc.tile_pool(name="aact", bufs=3) as aact:
        whh_sb = {}
        for d, wn in (("f", "wf_hh"), ("b", "wb_hh")):
            whh_sb[d] = aw.tile([128, 4, 4 * H], hdt, tag=f"whh_{d}", name=f"whh_{d}")
            nc.sync.dma_start(out=whh_sb[d], in_=t_[wn].rearrange("(k p) n -> p k n", p=128))

        hT = g["hT"]
        cst = g["cst"]

        def l1_step(d, s):
            Xd = g["Xf"] if d == "f" else g["Xb"]
            x_idx = s if d == "f" else T - 1 - s
            outT = g["fT"] if d == "f" else g["bT"]
            xt = ax.tile([BL, 4 * H], BF16, tag=f"xa_{d}", name=f"xa_{d}")
            nc.sync.dma_start(out=xt, in_=Xd[x_idx // 4][x_idx % 4])
            # gate quarters [g|i|f|o], each [BL, 512] in PSUM
            q = []
            for qi in range(4):
                ps = ag.tile([BL, H], F32, tag=f"g_{d}", name=f"g{qi}_{d}")
                nsl = slice(qi * H, (qi + 1) * H)
                nc.tensor.matmul(
                    ps, t_["id32_t"], xt[:, nsl], start=True, stop=False,
                )
                if use_fp8:
                    for kp in range(2):
                        nc.tensor.matmul(
                            ps, hT[d][:, 2 * kp:2 * kp + 2, :],
                            whh_sb[d][:, 2 * kp:2 * kp + 2, nsl],
                            start=False, stop=(kp == 1),
                            perf_mode=mybir.MatmulPerfMode.DoubleRow,
                        )
                else:
                    for kt in range(4):
                        nc.tensor.matmul(
                            ps, hT[d][:, kt], whh_sb[d][:, kt, nsl],
                            start=False, stop=(kt == 3),
                        )
                q.append(ps)
            # order [g|i|f|o]
            tg = aact.tile([BL, H], F32, tag=f"tg_{d}", name=f"tg_{d}")
            nc.scalar.activation(tg, q[0], AF.Tanh)
            si = aact.tile([BL, H], F32, tag=f"si_{d}", name=f"si_{d}")
            nc.scalar.activation(si, q[1], AF.Sigmoid)
            sf = aact.tile([BL, H], F32, tag=f"sf_{d}", name=f"sf_{d}")
            nc.scalar.activation(sf, q[2], AF.Sigmoid)
            so = aact.tile([BL, H], F32, tag=f"so_{d}", name=f"so_{d}")
            nc.scalar.activation(so, q[3], AF.Sigmoid)

            u = aact.tile([BL, H], F32, tag=f"u_{d}", name=f"u_{d}")
            nc.gpsimd.tensor_mul(u, si, tg)              # i * g
            v = aact.tile([BL, H], F32, tag=f"v_{d}", name=f"v_{d}")
            nc.vector.tensor_mul(v, sf, cst[d])          # f * c
            nc.vector.tensor_add(cst[d], u, v)           # c = u + v
            tc_ = aact.tile([BL, H], F32, tag=f"tc_{d}", name=f"tc_{d}")
            nc.scalar.activation(tc_, cst[d], AF.Tanh)
            h = aact.tile([BL, H], BF16, tag=f"h_{d}", name=f"h_{d}")
            nc.vector.tensor_mul(h, so, tc_)             # h = o * tanh(c)

            ptr = atr.tile([128, 4, 32], BF16, tag=f"tr_{d}", name=f"tr_{d}")
            for kt in range(4):
                nc.tensor.transpose(
                    ptr[:, kt], h[:, kt * 128:(kt + 1) * 128], t_["id32_t"],
                )
            nc.vector.tensor_copy(hT[d], ptr)
            nc.sync.dma_start(
                out=outT[s // 4][:, :, s % 4, :].rearrange("k p b -> p k b"),
                in_=hT[d],
            )

        for s in range(T):
            l1_step("f", s)
            l1_step("b", s)


def _emit_Xs(nc, tc, T, t_, g, use_fp8):
    """Xs[mt] = combined @ Ws_ih^T + bs."""
    hdt = F8 if use_fp8 else BF16
    MT = T // 4
    with tc.tile_pool(name="sw", bufs=1) as sw, \
         tc.tile_pool(name="sk", bufs=3) as sk, \
         tc.tile_pool(name="sp", bufs=2, space="PSUM") as sp, \
         tc.tile_pool(name="so_", bufs=3) as so_:
        wsih_sb = sw.tile([128, 8, 4 * H2], hdt, tag="wsih", name="wsih")
        nc.sync.dma_start(out=wsih_sb, in_=t_["ws_ih"].rearrange("(k p) n -> p k n", p=128))
        bs_sb = sw.tile([1, 4 * H2], BF16, tag="bs", name="bs")
        nc.sync.dma_start(out=bs_sb, in_=t_["bs_r"][:, :])

        for mt in range(MT):
            ck = sk.tile([128, 8, 4, 32], hdt, tag="ck", name="ck")
            for kt in range(8):
                src = g["fT"] if kt < 4 else g["bT"]
                nc.sync.dma_start(out=ck[:, kt], in_=src[mt][kt % 4])
            for half in range(2):
                ps = sp.tile([128, 2 * H2], F32, tag="ps", name="ps")
                for nh in range(4):
                    nsl_p = slice(nh * 512, (nh + 1) * 512)
                    nsl_w = slice(half * 2048 + nh * 512, half * 2048 + (nh + 1) * 512)
                    if use_fp8:
                        for kp in range(4):
                            nc.tensor.matmul(
                                ps[:, nsl_p],
                                ck[:, 2 * kp:2 * kp + 2].rearrange("p k t b -> p k (t b)"),
                                wsih_sb[:, 2 * kp:2 * kp + 2, nsl_w],
                                start=(kp == 0), stop=False,
                                perf_mode=mybir.MatmulPerfMode.DoubleRow,
                            )
                    else:
                        for kt in range(8):
                            nc.tensor.matmul(
                                ps[:, nsl_p],
                                ck[:, kt].rearrange("p t b -> p (t b)"),
                                wsih_sb[:, kt, nsl_w],
                                start=(kt == 0), stop=False,
                            )
                    nc.tensor.matmul(
                        ps[:, nsl_p], t_["ones_t"][:, :128], bs_sb[:, nsl_w],
                        start=False, stop=True,
                    )
                ot = so_.tile([128, 2 * H2], BF16, tag="ot", name="ot")
                nc.vector.tensor_copy(ot, ps)
                nc.sync.dma_start(
                    out=g["Xs"][mt][:, :, half * 2048:(half + 1) * 2048]
                        .rearrange("t b n -> (t b) n"),
                    in_=ot,
                )


def _emit_B(nc, tc, T, t_, g, use_fp8):
    """Second LSTM over combined; classifier on final h."""
    hdt = F8 if use_fp8 else BF16
    with tc.tile_pool(name="bw", bufs=1) as bw, \
         tc.tile_pool(name="bx", bufs=3) as bx, \
         tc.tile_pool(name="bg", bufs=3, space="PSUM") as bg, \
         tc.tile_pool(name="btr", bufs=1, space="PSUM") as btr, \
         tc.tile_pool(name="bact", bufs=2) as bact:
        wshh_sb = bw.tile([128, 8, 4 * H2], hdt, tag="wshh", name="wshh")
        nc.sync.dma_start(out=wshh_sb, in_=t_["ws_hh"].rearrange("(k p) n -> p k n", p=128))
        h2T, c2 = g["h2T"], g["c2"]

        for s in range(T):
            # quarters [g|i|f|o], each [BL, 1024]
            xs_t = bx.tile([BL, 4 * H2], BF16, tag="xb", name="xs_t")
            nc.sync.dma_start(out=xs_t, in_=g["Xs"][s // 4][s % 4])
            gq = []
            for qi in range(4):
                xt = xs_t[:, qi * H2:(qi + 1) * H2]
                ps = bg.tile([BL, H2], F32, tag="bg", name=f"bg{qi}")
                for nh in range(2):
                    psl = slice(nh * 512, (nh + 1) * 512)
                    nsl = slice(qi * H2 + nh * 512, qi * H2 + (nh + 1) * 512)
                    if use_fp8:
                        nc.tensor.matmul(
                            ps[:, psl], t_["id32_t"], xt[:, psl],
                            start=True, stop=False,
                        )
                        for kp in range(4):
                            nc.tensor.matmul(
                                ps[:, psl], h2T[:, 2 * kp:2 * kp + 2, :],
                                wshh_sb[:, 2 * kp:2 * kp + 2, nsl],
                                start=False, stop=(kp == 3),
                                perf_mode=mybir.MatmulPerfMode.DoubleRow,
                            )
                    else:
                        for kt in range(8):
                            nc.tensor.matmul(
                                ps[:, psl], h2T[:, kt], wshh_sb[:, kt, nsl],
                                start=(kt == 0), stop=(kt == 7),
                            )
                if use_fp8:
                    gq.append(ps)
                else:
                    q = bact.tile([BL, H2], F32, tag=f"gb{qi}", name=f"gb{qi}")
                    nc.vector.tensor_add(q, ps, xt)
                    gq.append(q)
            tg = bact.tile([BL, H2], F32, tag="tg2", name="tg2")
            nc.scalar.activation(tg, gq[0], AF.Tanh)
            si = bact.tile([BL, H2], F32, tag="si2", name="si2")
            nc.scalar.activation(si, gq[1], AF.Sigmoid)
            sf = bact.tile([BL, H2], F32, tag="sf2", name="sf2")
            nc.scalar.activation(sf, gq[2], AF.Sigmoid)
            so2 = bact.tile([BL, H2], F32, tag="so2", name="so2")
            nc.scalar.activation(so2, gq[3], AF.Sigmoid)

            u = bact.tile([BL, H2], F32, tag="u2", name="u2")
            nc.gpsimd.tensor_mul(u, si, tg)
            v = bact.tile([BL, H2], F32, tag="v2", name="v2")
            nc.vector.tensor_mul(v, sf, c2)
            nc.vector.tensor_add(c2, u, v)
            tc2 = bact.tile([BL, H2], F32, tag="tc2", name="tc2")
            nc.scalar.activation(tc2, c2, AF.Tanh)
            h2 = bact.tile([BL, H2], BF16, tag="h2", name="h2")
            nc.vector.tensor_mul(h2, so2, tc2)

            ptr = btr.tile([128, 8, 32], BF16, tag="tr2", name="tr2")
            for kt in range(8):
                nc.tensor.transpose(
                    ptr[:, kt], h2[:, kt * 128:(kt + 1) * 128], t_["id32_t"],
                )
            nc.vector.tensor_copy(h2T, ptr)

        # ---- classifier: out = sigmoid(h2 @ Wl^T + bl) ----
        wl_sb = bw.tile([128, 8, L], BF16, tag="wl", name="wl")
        nc.sync.dma_start(out=wl_sb, in_=t_["wl"].rearrange("(k p) n -> p k n", p=128))
        bl_sb = bw.tile([1, L], BF16, tag="bl", name="bl")
        nc.sync.dma_start(out=bl_sb, in_=t_["bl_r"][:, :])
        h2b = bact.tile([128, 8, 32], BF16, tag="h2b", name="h2b")
        nc.vector.tensor_copy(h2b, h2T)
        ps_o = btr.tile([BL, L], F32, tag="ps_o", name="ps_o")
        for kt in range(8):
            nc.tensor.matmul(
                ps_o, h2b[:, kt], wl_sb[:, kt],
                start=(kt == 0), stop=False,
            )
        nc.tensor.matmul(ps_o, t_["ones_t"][:, :BL], bl_sb, start=False, stop=True)
        o_sb = bact.tile([BL, L], F32, tag="o_sb", name="o_sb")
        nc.scalar.activation(o_sb, ps_o, AF.Sigmoid)
        nc.sync.dma_start(out=t_["out"][:, :], in_=o_sb)


def _build_nc(T: int, phases=ALL_PHASES, use_fp8=False):
    nc = bacc.Bacc(None, target_bir_lowering=False)
    hdt = F8 if use_fp8 else BF16

    t_ = {}
    t_["xT"] = nc.dram_tensor("xT", [D, T, BL], BF16, kind="ExternalInput")
    for name, shape in (
        ("wf_ih", [D, 4 * H]), ("wb_ih", [D, 4 * H]),
        ("bf_r", [1, 4 * H]), ("bb_r", [1, 4 * H]), ("bs_r", [1, 4 * H2]),
        ("wl", [H2, L]), ("bl_r", [1, L]),
        ("ones_r", [1, 128]), ("id32", [32, 32]),
    ):
        t_[name] = nc.dram_tensor(name, shape, BF16, kind="ExternalInput")
    for name, shape in (
        ("wf_hh", [H, 4 * H]), ("wb_hh", [H, 4 * H]), ("ws_hh", [H2, 4 * H2]),
        ("ws_ih", [H2, 4 * H2]),
    ):
        t_[name] = nc.dram_tensor(name, shape, hdt, kind="ExternalInput")
    if use_fp8:
        t_["id32_8"] = nc.dram_tensor("id32_8", [32, 32], F8, kind="ExternalInput")
    t_["out"] = nc.dram_tensor("out", [BL, L], F32, kind="ExternalOutput")

    with tile.TileContext(nc) as tc:
        from contextlib import ExitStack
        with ExitStack() as ctx:
            ec = ctx.enter_context
            dram = ec(tc.tile_pool(name="dram", bufs=1, space="DRAM"))
            const = ec(tc.tile_pool(name="const", bufs=1))
            state = ec(tc.tile_pool(name="state", bufs=1))

            MT = T // 4
            g = {}
            g["Xf"] = [dram.tile([4, BL, 4 * H], BF16, tag=f"Xf{m}", name=f"Xf{m}") for m in range(MT)]
            g["Xb"] = [dram.tile([4, BL, 4 * H], BF16, tag=f"Xb{m}", name=f"Xb{m}") for m in range(MT)]
            g["Xs"] = [dram.tile([4, BL, 4 * H2], BF16, tag=f"Xs{m}", name=f"Xs{m}") for m in range(MT)]
            g["fT"] = [dram.tile([4, 128, 4, 32], hdt, tag=f"fT{m}", name=f"fT{m}") for m in range(MT)]
            g["bT"] = [dram.tile([4, 128, 4, 32], hdt, tag=f"bT{m}", name=f"bT{m}") for m in range(MT)]

            ones_t = const.tile([1, 128], BF16, tag="ones_t", name="ones_t")
            nc.sync.dma_start(out=ones_t, in_=t_["ones_r"][:, :])
            id32_t = const.tile([32, 32], BF16, tag="id32_t", name="id32_t")
            nc.sync.dma_start(out=id32_t, in_=t_["id32"][:, :])
            t_["ones_t"], t_["id32_t"] = ones_t, id32_t
            t_["id32_t8"] = None
            if use_fp8:
                id32_t8 = const.tile([32, 32], F8, tag="id32_t8", name="id32_t8")
                nc.sync.dma_start(out=id32_t8, in_=t_["id32_8"][:, :])
                t_["id32_t8"] = id32_t8

            g["hT"] = {}
            g["cst"] = {}
            for d in ("f", "b"):
                g["hT"][d] = state.tile([128, 4, 32], hdt, tag=f"hT_{d}", name=f"hT_{d}")
                nc.vector.memset(g["hT"][d], 0.0)
                g["cst"][d] = state.tile([BL, H], F32, tag=f"c_{d}", name=f"c_{d}")
                nc.vector.memset(g["cst"][d], 0.0)
            g["h2T"] = state.tile([128, 8, 32], hdt, tag="h2T", name="h2T")
            nc.vector.memset(g["h2T"], 0.0)
            g["c2"] = state.tile([BL, H2], F32, tag="c2", name="c2")
            nc.vector.memset(g["c2"], 0.0)

            if "ih" in phases:
                _emit_ih(nc, tc, T, t_, g)
            if "A" in phases:
                _emit_A(nc, tc, T, t_, g, use_fp8)
            if "Xs" in phases:
                _emit_Xs(nc, tc, T, t_, g, use_fp8)
            if "B" in phases:
                _emit_B(nc, tc, T, t_, g, use_fp8)
            else:
                o_sb = const.tile([BL, L], F32, tag="o0", name="o0")
                nc.vector.memset(o_sb, 0.0)
                nc.sync.dma_start(out=t_["out"][:, :], in_=o_sb)
    nc.compile()
    return nc


_NC_CACHE = {}
USE_FP8 = True


def _get_nc(T, use_fp8=None):
    if use_fp8 is None:
        use_fp8 = USE_FP8
    key = (T, use_fp8)
    if key not in _NC_CACHE:
        _NC_CACHE[key] = _build_nc(T, use_fp8=use_fp8)
    return _NC_CACHE[key]


def _bf16(a):
    return np.ascontiguousarray(np.asarray(a, dtype=np.float32)).astype(ml_dtypes.bfloat16)


def _f8(a):
    a = np.clip(np.asarray(a, dtype=np.float32), -240.0, 240.0)
    return np.ascontiguousarray(a).astype(ml_dtypes.float8_e4m3)


def _prep_weights(Wf_ih, Wf_hh, bf, Wb_ih, Wb_hh, bb, Ws_ih, Ws_hh, bs, Wl, bl,
                  use_fp8=None):
    if use_fp8 is None:
        use_fp8 = USE_FP8
    # gate reorder [i|f|g|o] -> [g|i|f|o]
    r1 = np.r_[2 * H:3 * H, 0:H, H:2 * H, 3 * H:4 * H]
    r2 = np.r_[2 * H2:3 * H2, 0:H2, H2:2 * H2, 3 * H2:4 * H2]
    whh = _f8 if use_fp8 else _bf16
    m = {
        "wf_ih": _bf16(np.asarray(Wf_ih)[r1].T),
        "wf_hh": whh(np.asarray(Wf_hh)[r1].T),
        "wb_ih": _bf16(np.asarray(Wb_ih)[r1].T),
        "wb_hh": whh(np.asarray(Wb_hh)[r1].T),
        "ws_ih": whh(np.asarray(Ws_ih)[r2].T),
        "ws_hh": whh(np.asarray(Ws_hh)[r2].T),
        "bf_r": _bf16(np.asarray(bf)[r1][None, :]),
        "bb_r": _bf16(np.asarray(bb)[r1][None, :]),
        "bs_r": _bf16(np.asarray(bs)[r2][None, :]),
        "wl": _bf16(np.asarray(Wl).T),
        "bl_r": _bf16(np.asarray(bl)[None, :]),
        "ones_r": _bf16(np.ones((1, 128), np.float32)),
        "id32": _bf16(np.eye(32, dtype=np.float32)),
    }
    if use_fp8:
        m["id32_8"] = _f8(np.eye(32, dtype=np.float32))
    return m


def input_specs(T):
    """(name, shape, dtype) for every external input — used by test.py's
    trivial-baseline kernel so transfer volume matches."""
    hdt = F8 if USE_FP8 else BF16
    specs = [
        ("xT", [D, T, BL], BF16), ("wf_ih", [D, 4 * H], BF16),
        ("wb_ih", [D, 4 * H], BF16), ("bf_r", [1, 4 * H], BF16),
        ("bb_r", [1, 4 * H], BF16), ("bs_r", [1, 4 * H2], BF16),
        ("wl", [H2, L], BF16), ("bl_r", [1, L], BF16),
        ("ones_r", [1, 128], BF16), ("id32", [32, 32], BF16),
        ("wf_hh", [H, 4 * H], hdt), ("wb_hh", [H, 4 * H], hdt),
        ("ws_hh", [H2, 4 * H2], hdt), ("ws_ih", [H2, 4 * H2], hdt),
    ]
    if USE_FP8:
        specs.append(("id32_8", [32, 32], F8))
    return specs


def make_in_maps(inputs):
    x = np.asarray(inputs["x"], dtype=np.float32)
    wmap = _prep_weights(
        inputs["Wf_ih"], inputs["Wf_hh"], inputs["bf"],
        inputs["Wb_ih"], inputs["Wb_hh"], inputs["bb"],
        inputs["Ws_ih"], inputs["Ws_hh"], inputs["bs"],
        inputs["Wl"], inputs["bl"])
    in_maps = []
    for c in range(NCORES):
        m = dict(wmap)
        m["xT"] = _bf16(x[c * BL:(c + 1) * BL].transpose(2, 1, 0))
        in_maps.append(m)
    return in_maps


def kernel(x, Wf_ih, Wf_hh, bf, Wb_ih, Wb_hh, bb, Ws_ih, Ws_hh, bs, Wl, bl):
    x = np.asarray(x, dtype=np.float32)
    T = x.shape[1]
    nc = _get_nc(T)
    wmap = _prep_weights(Wf_ih, Wf_hh, bf, Wb_ih, Wb_hh, bb, Ws_ih, Ws_hh, bs, Wl, bl)
    in_maps = []
    for c in range(NCORES):
        xc = x[c * BL:(c + 1) * BL]            # [BL, T, D]
        m = dict(wmap)
        m["xT"] = _bf16(xc.transpose(2, 1, 0))  # [D, T, BL]
        in_maps.append(m)
    res = run_bass_kernel_spmd(nc, in_maps, list(range(NCORES)))
    return np.concatenate([res.results[c]["out"] for c in range(NCORES)], axis=0)


if __name__ == "__main__":
    rng = np.random.default_rng(0)
    T = int(sys.argv[1]) if len(sys.argv) > 1 else 8
    if len(sys.argv) > 2 and sys.argv[2] == "bf16":
        USE_FP8 = False
    ins = {
        "x": rng.standard_normal((B, T, D), dtype=np.float32),
        "Wf_ih": rng.standard_normal((4 * H, D), dtype=np.float32) * 0.05,
        "Wf_hh": rng.standard_normal((4 * H, H), dtype=np.float32) * 0.04,
        "bf": np.zeros(4 * H, np.float32),
        "Wb_ih": rng.standard_normal((4 * H, D), dtype=np.float32) * 0.05,
        "Wb_hh": rng.standard_normal((4 * H, H), dtype=np.float32) * 0.04,
        "bb": np.zeros(4 * H, np.float32),
        "Ws_ih": rng.standard_normal((4 * H2, H2), dtype=np.float32) * 0.03,
        "Ws_hh": rng.standard_normal((4 * H2, H2), dtype=np.float32) * 0.03,
        "bs": np.zeros(4 * H2, np.float32),
        "Wl": rng.standard_normal((L, H2), dtype=np.float32) * 0.04,
        "bl": np.zeros(L, np.float32),
    }
    got = kernel(**ins)

    def sigmoid(z):
        return 1.0 / (1.0 + np.exp(-z))

    def scan(xs, Wih, Whh, bvec):
        Tn, Bn, _ = xs.shape
        Hh = Whh.shape[1]
        h = np.zeros((Bn, Hh), np.float32)
        c = np.zeros((Bn, Hh), np.float32)
        hs = []
        for t in range(Tn):
            gg = xs[t] @ Wih.T + h @ Whh.T + bvec
            i, f, ge, o = np.split(gg, 4, axis=-1)
            c = sigmoid(f) * c + sigmoid(i) * np.tanh(ge)
            h = sigmoid(o) * np.tanh(c)
            hs.append(h.copy())
        return np.stack(hs), h

    xs = np.swapaxes(ins["x"], 0, 1)
    fseq, _ = scan(xs, ins["Wf_ih"], ins["Wf_hh"], ins["bf"])
    bseq, _ = scan(xs[::-1], ins["Wb_ih"], ins["Wb_hh"], ins["bb"])
    comb = np.concatenate([fseq, bseq], -1)
    _, hs = scan(comb, ins["Ws_ih"], ins["Ws_hh"], ins["bs"])
    ref = sigmoid(hs @ ins["Wl"].T + ins["bl"])
    rel = np.abs(got - ref) / np.maximum(np.abs(ref), 1e-6)
    print(f"T={T} fp8={USE_FP8}: max rel {rel.max():.3e}  mean rel {rel.mean():.3e}")



# revision 3
# speedup vs baseline: 16.5645x; 16.5645x over previous
"""BiLSTM classifier Trainium2 kernel, v3.

Data-parallel over batch across 8 NeuronCores (BL=32/core, replicated
weights). Single-core program per core; no collectives.

Layout (walrus-valid: DoubleRow matmuls must write PSUM partition 0):
- L1 gates per quarter in PSUM [32, 1024]: cols 0:512 fwd, 512:1024 bwd.
  One activation covers both directions; c-chain is [32, 1024].
- L2 gates per quarter in PSUM [32, 1024]: cols = two hidden halves.
- Per-gate emission order [f, g, i, o]: sigmoid(f) -> v=f*c starts
  while later gate matmuls still stream.
- h^T history kept in SBUF per-m-tile ring tiles (hTs); Xs phase reads
  its stationary operands straight from them (no DRAM round trip).
- bwd-direction X precompute emitted in descending x-mt order so the
  bwd recurrence (which consumes x reversed) starts immediately.
- ih/A/Xs/B emitted interleaved (software pipeline) so the Tile
  scheduler sees instructions in near-execution order.
- hh/ws weights fp8-e4m3 DoubleRow (0.5 cyc/row); ih bf16; gate
  activations fp16 (DVE 2x where eligible); c-state f32.
"""

import sys

sys.path.insert(0, "/opt/trn_rl_repo")

import numpy as np
import ml_dtypes

import concourse.bass as bass
import concourse.mybir as mybir
import concourse.tile as tile
from concourse import bacc
from concourse.bass_utils import run_bass_kernel_spmd

AF = mybir.ActivationFunctionType
BF16 = mybir.dt.bfloat16
F16 = mybir.dt.float16
F32 = mybir.dt.float32
F8 = mybir.dt.float8e4
DR = mybir.MatmulPerfMode.DoubleRow

B, D, H = 256, 256, 512
H2 = 2 * H
G1 = 4 * H           # 2048 = L1 gate width per direction
G2 = 4 * H2          # 4096 = L2 gate width
L = 2
NCORES = 8
BL = B // NCORES     # 32 local batch

# gate-quarter emission order: f first (so v=f*c starts early), o last
QORDER = (2, 0, 1, 3)  # col order is [g|i|f|o]


def _ih_iter(nc, pools, T, t_, g, i):
    """One ih iteration: Xf[i] (fwd) + Xb[MT-1-i] (bwd, descending)."""
    MT = T // 4
    xT = t_["xT"]
    ihx, ihp, iho = pools["ihx"], pools["ihp"], pools["iho"]
    for d in ("f", "b"):
        mt = i if d == "f" else MT - 1 - i
        t0 = mt * 4
        xt = ihx.tile([128, 2, 4, 32], BF16, tag=f"xt_{d}", name=f"xt_{d}")
        nc.sync.dma_start(
            out=xt,
            in_=xT.rearrange("(k p) t b -> p k t b", p=128)[:, :, t0:t0 + 4, :],
        )
        Xd = g["Xf"] if d == "f" else g["Xb"]
        wih = t_[f"w{d}_ih_sb"]
        bih = t_[f"b{d}_sb"]
        for nh in range(4):
            nsl = slice(nh * 512, (nh + 1) * 512)
            ps = ihp.tile([128, 512], F32, tag="ps", name="ps")
            for kt in range(2):
                nc.tensor.matmul(
                    ps, xt[:, kt].rearrange("p t b -> p (t b)"),
                    wih[:, kt, nsl], start=(kt == 0), stop=False,
                )
            nc.tensor.matmul(
                ps, t_["ones_t"][:, :128], bih[:, nsl],
                start=False, stop=True,
            )
            ot = iho.tile([128, 512], BF16, tag="ot", name="ot")
            nc.vector.tensor_copy(ot, ps)
            nc.sync.dma_start(
                out=Xd[mt][:, :, nsl].rearrange("t b n -> (t b) n"),
                in_=ot,
            )


def _a_step(nc, pools, T, t_, g, s):
    """One L1 step; fwd at cols 0:512, bwd at cols 512:1024 per quarter."""
    ax, ag, atr, ah, aact = (pools[k] for k in ("ax", "ag", "atr", "ah", "aact"))
    c1 = g["c1"]
    if s % 4 == 0:
        # [128, dir, kt, st, b] fp8 - per-m-tile h^T history ring
        g["hTs"][s // 4] = ah.tile(
            [128, 2, 4, 4, 32], F8, tag="hTs", name=f"hTs{s//4}")
    hTs = g["hTs"][s // 4]

    def prev_ap(bi, kp):
        if s == 0:
            return g["hT0"][:, bi, 2 * kp:2 * kp + 2, :]
        return g["hTs"][(s - 1) // 4][:, bi, 2 * kp:2 * kp + 2, (s - 1) % 4, :]

    xfb = ax.tile([32, 2 * G1], BF16, tag="xfb", name="xfb")
    nc.sync.dma_start(out=xfb[:, 0:G1], in_=g["Xf"][s // 4][s % 4])
    sb = T - 1 - s
    nc.sync.dma_start(out=xfb[:, G1:2 * G1], in_=g["Xb"][sb // 4][sb % 4])

    acts = {}
    for q in QORDER:
        nm = {2: "sf", 0: "tg", 1: "si", 3: "so"}[q]
        acts[nm] = aact.tile([32, 1024], F16, tag=nm, name=nm)
    for q in QORDER:
        nm = {2: "sf", 0: "tg", 1: "si", 3: "so"}[q]
        fn = AF.Tanh if q == 0 else AF.Sigmoid
        for bi, d in enumerate(("f", "b")):
            gp = ag.tile([32, 512], F32, tag="gq", name=f"gq{q}{d}")
            nc.tensor.matmul(
                gp, t_["id32_t"],
                xfb[:, bi * G1 + q * 512: bi * G1 + (q + 1) * 512],
                start=True, stop=False,
            )
            whh = t_[f"w{d}_hh_sb"]
            for kp in range(2):
                nc.tensor.matmul(
                    gp, prev_ap(bi, kp),
                    whh[:, 2 * kp:2 * kp + 2, q * 512:(q + 1) * 512],
                    start=False, stop=(kp == 1), perf_mode=DR,
                )
            nc.scalar.activation(
                acts[nm][:, 512 * bi:512 * bi + 512], gp, fn)
    sf, tg, si, so = acts["sf"], acts["tg"], acts["si"], acts["so"]

    v = aact.tile([32, 1024], F32, tag="v", name="v")
    nc.vector.tensor_mul(v, sf, c1)
    u = aact.tile([32, 1024], F16, tag="u", name="u")
    nc.vector.tensor_mul(u, si, tg)
    nc.vector.tensor_add(c1, u, v)
    tcl = aact.tile([32, 1024], F16, tag="tc", name="tc")
    nc.scalar.activation(tcl, c1, AF.Tanh)
    h = aact.tile([32, 1024], BF16, tag="h", name="h")
    nc.vector.tensor_mul(h, so, tcl)

    ptr = atr.tile([128, 8, 32], BF16, tag="ptr", name="ptr")
    for ci in range(8):  # cols 0:512 fwd (kt=ci), 512:1024 bwd (kt=ci-4)
        nc.tensor.transpose(
            ptr[:, ci], h[:, ci * 128:(ci + 1) * 128], t_["id32_t"],
        )
    for bi in range(2):
        nc.vector.tensor_copy(
            hTs[:, bi, :, s % 4, :], ptr[:, 4 * bi:4 * bi + 4, :])


def _xs_mt(nc, pools, T, t_, g, mt):
    """Xs[mt] = combined @ Ws_ih^T + bs (natural col order)."""
    sp, so_ = pools["sp"], pools["so_"]
    wsih = t_["ws_ih_sb"]
    hTs = g["hTs"][mt]
    for chunk in range(8):
        csl = slice(chunk * 512, (chunk + 1) * 512)
        ps = sp.tile([128, 512], F32, tag="ps", name="ps")
        # stationary k-slab pairs: (f0,f1),(f2,f3),(b0,b1),(b2,b3)
        for kp in range(4):
            bi = kp // 2
            ko = (kp % 2) * 2
            lhs = hTs[:, bi, ko:ko + 2, :, :].rearrange("p k t b -> p k (t b)")
            nc.tensor.matmul(
                ps, lhs, wsih[:, 2 * kp:2 * kp + 2, csl],
                start=(kp == 0), stop=False, perf_mode=DR,
            )
        nc.tensor.matmul(
            ps, t_["ones_t"][:, :128], t_["bs_sb"][:, csl],
            start=False, stop=True,
        )
        ot = so_.tile([128, 512], BF16, tag="ot", name="ot")
        nc.vector.tensor_copy(ot, ps)
        nc.sync.dma_start(
            out=g["Xs"][mt][:, :, csl].rearrange("t b n -> (t b) n"),
            in_=ot,
        )


def _b_step(nc, pools, T, t_, g, s):
    """One L2 step; hidden halves at cols 0:512 / 512:1024 per quarter."""
    bx, bg, btr, bact = (pools[k] for k in ("bx", "bg", "btr", "bact"))
    wshh = t_["ws_hh_sb"]
    h2T, c2 = g["h2T"], g["c2"]
    xs2 = bx.tile([32, G2], BF16, tag="xs2", name="xs2")
    nc.sync.dma_start(out=xs2, in_=g["Xs"][s // 4][s % 4])
    acts = {}
    for q in QORDER:
        nm = {2: "sf2", 0: "tg2", 1: "si2", 3: "so2"}[q]
        acts[nm] = bact.tile([32, 1024], F16, tag=nm, name=nm)
    for q in QORDER:
        nm = {2: "sf2", 0: "tg2", 1: "si2", 3: "so2"}[q]
        fn = AF.Tanh if q == 0 else AF.Sigmoid
        for j in range(2):
            wsl = slice(1024 * q + 512 * j, 1024 * q + 512 * j + 512)
            gp = bg.tile([32, 512], F32, tag="gq2", name=f"gq2_{q}{j}")
            nc.tensor.matmul(
                gp, t_["id32_t"], xs2[:, wsl],
                start=True, stop=False,
            )
            for kp in range(4):
                nc.tensor.matmul(
                    gp, h2T[:, 2 * kp:2 * kp + 2, :],
                    wshh[:, 2 * kp:2 * kp + 2, wsl],
                    start=False, stop=(kp == 3), perf_mode=DR,
                )
            nc.scalar.activation(
                acts[nm][:, 512 * j:512 * j + 512], gp, fn)
    sf, tg, si, so = acts["sf2"], acts["tg2"], acts["si2"], acts["so2"]

    v = bact.tile([32, 1024], F32, tag="v2", name="v2")
    nc.vector.tensor_mul(v, sf, c2)
    u = bact.tile([32, 1024], F16, tag="u2", name="u2")
    nc.vector.tensor_mul(u, si, tg)
    nc.vector.tensor_add(c2, u, v)
    tcl = bact.tile([32, 1024], F16, tag="tc2", name="tc2")
    nc.scalar.activation(tcl, c2, AF.Tanh)
    h2 = bact.tile([32, 1024], BF16, tag="h2", name="h2")
    nc.vector.tensor_mul(h2, so, tcl)

    ptr = btr.tile([128, 8, 32], BF16, tag="ptr2", name="ptr2")
    for ci in range(8):  # cols are h2-dims in natural (slab) order
        nc.tensor.transpose(
            ptr[:, ci], h2[:, ci * 128:(ci + 1) * 128], t_["id32_t"],
        )
    nc.vector.tensor_copy(h2T, ptr)


def _classifier(nc, pools, t_, g):
    bg, bact = pools["bg"], pools["bact"]
    h2b = bact.tile([128, 8, 32], BF16, tag="h2b", name="h2b")
    nc.vector.tensor_copy(h2b, g["h2T"])
    ps_full = bg.tile([32, 512], F32, tag="gq2", name="ps_cls")
    ps_o = ps_full[0:BL, 0:L]
    for kt in range(8):
        nc.tensor.matmul(
            ps_o, h2b[:, kt], t_["wl_sb"][:, kt],
            start=(kt == 0), stop=False,
        )
    nc.tensor.matmul(
        ps_o, t_["ones_t"][:, :BL], t_["bl_sb"], start=False, stop=True)
    o_sb = bact.tile([BL, L], F32, tag="o_sb", name="o_sb")
    nc.scalar.activation(o_sb, ps_o, AF.Sigmoid)
    nc.sync.dma_start(out=t_["out"][:, :], in_=o_sb)


def _build_nc(T: int, use_fp8=True):
    assert use_fp8, "v3 kernel is fp8-DR only"
    nc = bacc.Bacc(None, target_bir_lowering=False)

    t_ = {}
    t_["xT"] = nc.dram_tensor("xT", [D, T, BL], BF16, kind="ExternalInput")
    for name, shape in (
        ("wf_ih", [D, G1]), ("wb_ih", [D, G1]),
        ("bf_r", [1, G1]), ("bb_r", [1, G1]), ("bs_r", [1, G2]),
        ("wl", [H2, L]), ("bl_r", [1, L]),
        ("ones_r", [1, 128]), ("id32", [32, 32]),
    ):
        t_[name] = nc.dram_tensor(name, shape, BF16, kind="ExternalInput")
    for name, shape in (
        ("wf_hh", [H, G1]), ("wb_hh", [H, G1]),
        ("ws_hh", [H2, G2]), ("ws_ih", [H2, G2]),
    ):
        t_[name] = nc.dram_tensor(name, shape, F8, kind="ExternalInput")
    t_["out"] = nc.dram_tensor("out", [BL, L], F32, kind="ExternalOutput")

    with tile.TileContext(nc) as tc:
        from contextlib import ExitStack
        with ExitStack() as ctx:
            ec = ctx.enter_context
            dram = ec(tc.tile_pool(name="dram", bufs=1, space="DRAM"))
            const = ec(tc.tile_pool(name="const", bufs=1))
            wpool = ec(tc.tile_pool(name="wpool", bufs=1))
            state = ec(tc.tile_pool(name="state", bufs=1))

            MT = T // 4
            g = {"hTs": {}}
            g["Xf"] = [dram.tile([4, BL, G1], BF16, tag=f"Xf{m}", name=f"Xf{m}") for m in range(MT)]
            g["Xb"] = [dram.tile([4, BL, G1], BF16, tag=f"Xb{m}", name=f"Xb{m}") for m in range(MT)]
            g["Xs"] = [dram.tile([4, BL, G2], BF16, tag=f"Xs{m}", name=f"Xs{m}") for m in range(MT)]

            ones_t = const.tile([1, 128], BF16, tag="ones_t", name="ones_t")
            nc.sync.dma_start(out=ones_t, in_=t_["ones_r"][:, :])
            id32_t = const.tile([32, 32], BF16, tag="id32_t", name="id32_t")
            nc.sync.dma_start(out=id32_t, in_=t_["id32"][:, :])
            t_["ones_t"], t_["id32_t"] = ones_t, id32_t

            # weights to SBUF
            for d in ("f", "b"):
                w = wpool.tile([128, 2, G1], BF16, tag=f"w{d}ih", name=f"w{d}ih")
                nc.sync.dma_start(out=w, in_=t_[f"w{d}_ih"].rearrange("(k p) n -> p k n", p=128))
                t_[f"w{d}_ih_sb"] = w
                w = wpool.tile([128, 4, G1], F8, tag=f"w{d}hh", name=f"w{d}hh")
                nc.sync.dma_start(out=w, in_=t_[f"w{d}_hh"].rearrange("(k p) n -> p k n", p=128))
                t_[f"w{d}_hh_sb"] = w
                bt = wpool.tile([1, G1], BF16, tag=f"b{d}", name=f"b{d}")
                nc.sync.dma_start(out=bt, in_=t_[f"b{d}_r"][:, :])
                t_[f"b{d}_sb"] = bt
            for nm in ("ws_ih", "ws_hh"):
                w = wpool.tile([128, 8, G2], F8, tag=nm, name=nm)
                nc.sync.dma_start(out=w, in_=t_[nm].rearrange("(k p) n -> p k n", p=128))
                t_[f"{nm}_sb"] = w
            bs = wpool.tile([1, G2], BF16, tag="bs", name="bs")
            nc.sync.dma_start(out=bs, in_=t_["bs_r"][:, :])
            t_["bs_sb"] = bs
            wl = wpool.tile([128, 8, L], BF16, tag="wl", name="wl")
            nc.sync.dma_start(out=wl, in_=t_["wl"].rearrange("(k p) n -> p k n", p=128))
            t_["wl_sb"] = wl
            bl = wpool.tile([1, L], BF16, tag="bl", name="bl")
            nc.sync.dma_start(out=bl, in_=t_["bl_r"][:, :])
            t_["bl_sb"] = bl

            g["hT0"] = state.tile([128, 2, 4, 32], F8, tag="hT0", name="hT0")
            nc.vector.memset(g["hT0"], 0.0)
            g["c1"] = state.tile([32, 1024], F32, tag="c1", name="c1")
            nc.vector.memset(g["c1"], 0.0)
            g["h2T"] = state.tile([128, 8, 32], F8, tag="h2T", name="h2T")
            nc.vector.memset(g["h2T"], 0.0)
            g["c2"] = state.tile([32, 1024], F32, tag="c2", name="c2")
            nc.vector.memset(g["c2"], 0.0)

            pools = {}
            pools["ihx"] = ec(tc.tile_pool(name="ihx", bufs=3))
            pools["ihp"] = ec(tc.tile_pool(name="ihp", bufs=1, space="PSUM"))
            pools["iho"] = ec(tc.tile_pool(name="iho", bufs=3))
            pools["ax"] = ec(tc.tile_pool(name="ax", bufs=2))
            pools["ag"] = ec(tc.tile_pool(name="ag", bufs=2, space="PSUM"))
            pools["atr"] = ec(tc.tile_pool(name="atr", bufs=1, space="PSUM"))
            pools["ah"] = ec(tc.tile_pool(name="ah", bufs=3))
            pools["aact"] = ec(tc.tile_pool(name="aact", bufs=1))
            pools["sp"] = ec(tc.tile_pool(name="sp", bufs=1, space="PSUM"))
            pools["so_"] = ec(tc.tile_pool(name="so_", bufs=3))
            pools["bx"] = ec(tc.tile_pool(name="bx", bufs=2))
            pools["bg"] = ec(tc.tile_pool(name="bg", bufs=2, space="PSUM"))
            pools["btr"] = ec(tc.tile_pool(name="btr", bufs=1, space="PSUM"))
            pools["bact"] = ec(tc.tile_pool(name="bact", bufs=1))

            # software pipeline: ih(i) || A(4(i-1)..) || Xs(i-2) || B(4(i-2)..)
            for i in range(MT + 2):
                if i < MT:
                    _ih_iter(nc, pools, T, t_, g, i)
                if 1 <= i <= MT:
                    for st in range(4):
                        _a_step(nc, pools, T, t_, g, 4 * (i - 1) + st)
                if 2 <= i:
                    mt = i - 2
                    _xs_mt(nc, pools, T, t_, g, mt)
                    for st in range(4):
                        _b_step(nc, pools, T, t_, g, 4 * mt + st)
            _classifier(nc, pools, t_, g)
    nc.compile()
    return nc


_NC_CACHE = {}
USE_FP8 = True


def _get_nc(T, use_fp8=None):
    if T not in _NC_CACHE:
        _NC_CACHE[T] = _build_nc(T)
    return _NC_CACHE[T]


def _bf16(a):
    return np.ascontiguousarray(np.asarray(a, dtype=np.float32)).astype(ml_dtypes.bfloat16)


def _f8(a):
    a = np.clip(np.asarray(a, dtype=np.float32), -240.0, 240.0)
    return np.ascontiguousarray(a).astype(ml_dtypes.float8_e4m3)


def _prep_weights(Wf_ih, Wf_hh, bf, Wb_ih, Wb_hh, bb, Ws_ih, Ws_hh, bs, Wl, bl):
    # gate reorder [i|f|g|o] -> [g|i|f|o]
    r1 = np.r_[2 * H:3 * H, 0:H, H:2 * H, 3 * H:4 * H]
    r2 = np.r_[2 * H2:3 * H2, 0:H2, H2:2 * H2, 3 * H2:4 * H2]
    return {
        "wf_ih": _bf16(np.asarray(Wf_ih)[r1].T),
        "wf_hh": _f8(np.asarray(Wf_hh)[r1].T),
        "wb_ih": _bf16(np.asarray(Wb_ih)[r1].T),
        "wb_hh": _f8(np.asarray(Wb_hh)[r1].T),
        "ws_ih": _f8(np.asarray(Ws_ih)[r2].T),
        "ws_hh": _f8(np.asarray(Ws_hh)[r2].T),
        "bf_r": _bf16(np.asarray(bf)[r1][None, :]),
        "bb_r": _bf16(np.asarray(bb)[r1][None, :]),
        "bs_r": _bf16(np.asarray(bs)[r2][None, :]),
        "wl": _bf16(np.asarray(Wl).T),
        "bl_r": _bf16(np.asarray(bl)[None, :]),
        "ones_r": _bf16(np.ones((1, 128), np.float32)),
        "id32": _bf16(np.eye(32, dtype=np.float32)),
    }


def input_specs(T):
    return [
        ("xT", [D, T, BL], BF16), ("wf_ih", [D, G1], BF16),
        ("wb_ih", [D, G1], BF16), ("bf_r", [1, G1], BF16),
        ("bb_r", [1, G1], BF16), ("bs_r", [1, G2], BF16),
        ("wl", [H2, L], BF16), ("bl_r", [1, L], BF16),
        ("ones_r", [1, 128], BF16), ("id32", [32, 32], BF16),
        ("wf_hh", [H, G1], F8), ("wb_hh", [H, G1], F8),
        ("ws_hh", [H2, G2], F8), ("ws_ih", [H2, G2], F8),
    ]


def make_in_maps(inputs):
    x = np.asarray(inputs["x"], dtype=np.float32)
    wmap = _prep_weights(
        inputs["Wf_ih"], inputs["Wf_hh"], inputs["bf"],
        inputs["Wb_ih"], inputs["Wb_hh"], inputs["bb"],
        inputs["Ws_ih"], inputs["Ws_hh"], inputs["bs"],
        inputs["Wl"], inputs["bl"])
    in_maps = []
    for c in range(NCORES):
        m = dict(wmap)
        m["xT"] = _bf16(x[c * BL:(c + 1) * BL].transpose(2, 1, 0))
        in_maps.append(m)
    return in_maps


def kernel(x, Wf_ih, Wf_hh, bf, Wb_ih, Wb_hh, bb, Ws_ih, Ws_hh, bs, Wl, bl):
    x = np.asarray(x, dtype=np.float32)
    T = x.shape[1]
    nc = _get_nc(T)
    in_maps = make_in_maps(dict(
        x=x, Wf_ih=Wf_ih, Wf_hh=Wf_hh, bf=bf, Wb_ih=Wb_ih, Wb_hh=Wb_hh,
        bb=bb, Ws_ih=Ws_ih, Ws_hh=Ws_hh, bs=bs, Wl=Wl, bl=bl))
    res = run_bass_kernel_spmd(nc, in_maps, list(range(NCORES)))
    return np.concatenate([res.results[c]["out"] for c in range(NCORES)], axis=0)


if __name__ == "__main__":
    rng = np.random.default_rng(0)
    T = int(sys.argv[1]) if len(sys.argv) > 1 else 8
    ins = {
        "x": rng.standard_normal((B, T, D), dtype=np.float32),
        "Wf_ih": rng.standard_normal((4 * H, D), dtype=np.float32) * 0.05,
        "Wf_hh": rng.standard_normal((4 * H, H), dtype=np.float32) * 0.04,
        "bf": np.zeros(4 * H, np.float32),
        "Wb_ih": rng.standard_normal((4 * H, D), dtype=np.float32) * 0.05,
        "Wb_hh": rng.standard_normal((4 * H, H), dtype=np.float32) * 0.04,
        "bb": np.zeros(4 * H, np.float32),
        "Ws_ih": rng.standard_normal((4 * H2, H2), dtype=np.float32) * 0.03,
        "Ws_hh": rng.standard_normal((4 * H2, H2), dtype=np.float32) * 0.03,
        "bs": np.zeros(4 * H2, np.float32),
        "Wl": rng.standard_normal((L, H2), dtype=np.float32) * 0.04,
        "bl": np.zeros(L, np.float32),
    }
    got = kernel(**ins)

    def sigmoid(z):
        return 1.0 / (1.0 + np.exp(-z))

    def scan(xs, Wih, Whh, bvec):
        Tn, Bn, _ = xs.shape
        Hh = Whh.shape[1]
        h = np.zeros((Bn, Hh), np.float32)
        c = np.zeros((Bn, Hh), np.float32)
        hs = []
        for t in range(Tn):
            gg = xs[t] @ Wih.T + h @ Whh.T + bvec
            i, f, ge, o = np.split(gg, 4, axis=-1)
            c = sigmoid(f) * c + sigmoid(i) * np.tanh(ge)
            h = sigmoid(o) * np.tanh(c)
            hs.append(h.copy())
        return np.stack(hs), h

    xs = np.swapaxes(ins["x"], 0, 1)
    fseq, _ = scan(xs, ins["Wf_ih"], ins["Wf_hh"], ins["bf"])
    bseq, _ = scan(xs[::-1], ins["Wb_ih"], ins["Wb_hh"], ins["bb"])
    comb = np.concatenate([fseq, bseq], -1)
    _, hs = scan(comb, ins["Ws_ih"], ins["Ws_hh"], ins["bs"])
    ref = sigmoid(hs @ ins["Wl"].T + ins["bl"])
    rel = np.abs(got - ref) / np.maximum(np.abs(ref), 1e-6)
    print(f"T={T}: max rel {rel.max():.3e}  mean rel {rel.mean():.3e}")
